# revision 19
# baseline (speedup 1.0000x reference)
"""Trainium2 Bass kernel for ClipPairWiseLossAll.

loss = sum_{i<j} || relu(r_i - r_j) ||_2   with r = repr[GT], M=512, N=768.

Pair space is split into two exactly-composing parts (8 cores, SPMD,
one shared NEFF; per-core behavior lives in the DMA'd data):

COLUMN part (j < 192; 60% of pairs, long streams):
  For a fixed j, e[n, i] = max(rT[n, i] - rT[n, j], 0) over i in
  [16m, 512), m = j//16. The subtrahend is a per-partition scalar, so a
  single 4x tensor_scalar (op0=subtract with f32 scalar AP, op1=max 0)
  computes sub+relu in one pass. Core c owns j in {16m+c, 16m+15-c},
  m < 12 -> 24 "A-slots". The i <= j sub-range is computed (relu
  garbage) and killed by a final mask.

DIAGONAL part (pairs (t, t+o), o < 320, t >= 192; short streams):
  Baseline-style: core c owns o in {16k+c+1, 16k+16-c}, k < 20 -> 40
  "D-slots". The per-core shift lives in rtab (rT shifted left by
  192+delta, HUGE-padded so rounded-up tails relu to exactly 0), so the
  device slices at uniform offset 16k. One 2x tensor_tensor sub + one
  4x tensor_scalar relu per k covers both slots and all 6 chunks.

Squares: ACT Square -> fp8 (PE DoubleRow) for the big groups; DVE
tensor_mul -> bf16 (2x) for the ten smallest diagonal groups, which
balances DVE and ACT at ~82us each. Reduction matmuls have 32 output
rows, so slots are spread over PSUM column-groups 0..2 (tile_position
col-tiling; quadrant 3 has a HW bug) and consecutive groups execute
concurrently in disjoint 32-column array strips. One-hot lhsT banks are
synthesized on-device from a single [:, 32] = 1 column via shifted
views. Tail per parity bank: mask multiply, ACT Sqrt with fused
row-sum; host adds the partials.
"""

import numpy as np

M = 512
N = 768
P = 128
NCH = N // P  # 6
NCORES = 8
NR = 32  # rows per col-group
NG = 3  # col-groups used (0..2; quadrant 3 is buggy)
NPS = NG * NR  # ps partitions per parity bank
NMA = 12  # column groups (A), m = 0..11
NKD = 20  # diagonal groups (D), k = 0..19
TMIN = 192  # diagonal part covers t >= TMIN (j >= 192)
NOUT = 2 * NPS  # output rows (two parity banks)

# groups whose square runs on DVE (bf16 tensor_mul, 2x) with bf16 matmuls
SQ_DVE_STEPS = frozenset(("D", k) for k in range(3, 20, 2))

HUGE = 3.0e38

_PROG = {}


def _emit_order():
    """Big groups first (pipeline fills while rtab streams in), smallest
    groups last so the drain chain at kernel end is short. Leading A-fleet
    groups only need bias+rt, which DMA first."""
    order = [("A", 0), ("A", 1), ("A", 2)]
    dk = list(range(NKD))
    am = list(range(3, NMA))
    while dk or am:
        if dk:
            order.append(("D", dk.pop(0)))
        if dk:
            order.append(("D", dk.pop(0)))
        if am:
            order.append(("A", am.pop(0)))
    return order


def _row_map():
    """step_i, sl -> (parity, colgroup, row-in-group).

    A-slots go to col-group 0 (fp8 DoubleRow needs dst partition 0);
    D slot 0 -> col-group 1, slot 1 -> col-group 2 (bf16 matmuls).
    """
    amap = {}
    a_cnt = [0, 0]
    d_cnt = [0, 0]
    for step_i, (kind, idx) in enumerate(_emit_order()):
        par = step_i % 2
        if kind == "A":
            amap[(step_i, 0)] = (par, 0, 2 * a_cnt[par])
            amap[(step_i, 1)] = (par, 0, 2 * a_cnt[par] + 1)
            a_cnt[par] += 1
        else:
            amap[(step_i, 0)] = (par, 1, d_cnt[par])
            amap[(step_i, 1)] = (par, 2, d_cnt[par])
            d_cnt[par] += 1
    return amap


def _build_program():
    if "nc" in _PROG:
        return _PROG["nc"]

    from contextlib import ExitStack

    import concourse.bass as bass
    import concourse.bacc as bacc
    import concourse.tile as tile
    from concourse import mybir

    AOT = mybir.AluOpType
    AFT = mybir.ActivationFunctionType
    bf16 = mybir.dt.bfloat16
    fp8 = mybir.dt.float8e4
    f32 = mybir.dt.float32

    nc = bacc.Bacc(
        "TRN2",
        target_bir_lowering=False,
        debug=False,
        enable_asserts=False,
        num_devices=NCORES,
    )

    LD_MAX = 320
    rt_d = nc.dram_tensor("rt", [P, NCH * M], bf16, kind="ExternalInput")
    rtab_d = nc.dram_tensor("rtab", [P, 2 * NCH * LD_MAX], bf16, kind="ExternalInput")
    bias_d = nc.dram_tensor("bias", [P, NCH * 2 * NMA], f32, kind="ExternalInput")
    mask_d = nc.dram_tensor("mask", [2 * NPS, M], f32, kind="ExternalInput")
    out_d = nc.dram_tensor("out", [NOUT, 1], f32, kind="ExternalOutput")

    with ExitStack() as ctx:
        tc = ctx.enter_context(tile.TileContext(nc))
        singles = ctx.enter_context(tc.tile_pool(name="singles", bufs=1))
        epool = ctx.enter_context(tc.tile_pool(name="e", bufs=5))
        e2pool = ctx.enter_context(tc.tile_pool(name="e2", bufs=5))
        pspool = ctx.enter_context(tc.tile_pool(name="ps", bufs=1, space="PSUM"))

        rt_sb = singles.tile([P, NCH, M], bf16)
        rtab_sb = singles.tile([P, 2, NCH, LD_MAX], bf16)
        bias_sb = singles.tile([P, NCH, 2 * NMA], f32)
        mask0 = singles.tile([NPS, M], f32)
        mask1 = singles.tile([NPS, M], f32)

        rt_view = rt_d.ap().rearrange("p (c t) -> p c t", c=NCH)
        rtab_view = rtab_d.ap().rearrange("p (s c t) -> p s c t", s=2, c=NCH)

        # sync (hw) queue in need-order: leading A-groups want bias + all
        # of rt; the first D-group (k=0) reads all of rtab
        nc.sync.dma_start(
            out=bias_sb, in_=bias_d.ap().rearrange("p (c s) -> p c s", c=NCH)
        )
        nc.sync.dma_start(out=rt_sb, in_=rt_view)
        nc.sync.dma_start(out=rtab_sb, in_=rtab_view)
        # late inputs on the gpsimd queue
        nc.gpsimd.dma_start(out=mask0, in_=mask_d.ap()[0:NPS, :])
        nc.gpsimd.dma_start(out=mask1, in_=mask_d.ap()[NPS : 2 * NPS, :])

        # one-hot lhsT banks: col rr of view [.., NR-rr : 2*NR-rr] is 1
        oh8 = singles.tile([P, 2, 2 * NR], fp8)
        nc.vector.memset(oh8, 0.0)
        nc.vector.memset(oh8[:, :, NR : NR + 1], 1.0)
        ohb = singles.tile([P, 2 * NR], bf16)
        nc.vector.memset(ohb, 0.0)
        nc.vector.memset(ohb[:, NR : NR + 1], 1.0)

        ps0 = pspool.tile([NPS, M], f32)
        ps1 = pspool.tile([NPS, M], f32)
        nc.vector.memset(ps0, 0.0)
        nc.vector.memset(ps1, 0.0)
        pss = [ps0, ps1]

        rowmap = _row_map()
        for step_i, (kind, idx) in enumerate(_emit_order()):
            ps = pss[step_i % 2]
            e_t = epool.tile([P, 2, NCH, M], bf16, tag="e")

            if kind == "A":
                m = idx
                L = M - 16 * m
                for sl in range(2):
                    s = 2 * m + sl
                    for c in range(NCH):
                        nc.vector.tensor_scalar(
                            out=e_t[:, sl, c, 0:L],
                            in0=rt_sb[:, c, 16 * m : M],
                            scalar1=bias_sb[:, c, s : s + 1],
                            scalar2=0.0,
                            op0=AOT.subtract,
                            op1=AOT.max,
                        )
            else:
                k = idx
                L = LD_MAX - 16 * k
                in0s = rt_sb[:, :, TMIN : TMIN + L]
                in0 = bass.AP(
                    tensor=in0s.tensor,
                    offset=in0s.offset,
                    ap=[in0s.ap[0], [0, 2], in0s.ap[1], in0s.ap[2]],
                )
                nc.vector.tensor_sub(
                    e_t[:, :, :, 0:L],
                    in0,
                    rtab_sb[:, :, :, 16 * k : 16 * k + L],
                )
                nc.vector.tensor_scalar(
                    out=e_t[:, :, :, 0:L],
                    in0=e_t[:, :, :, 0:L],
                    scalar1=0.0,
                    scalar2=None,
                    op0=AOT.max,
                )

            if kind == "D":
                # bf16 e2 (ACT for k<10, DVE 2x for k>=10); bf16 matmuls in
                # col-groups 1 (slot 0) and 2 (slot 1)
                e2b = e2pool.tile([P, 2, NCH, M], bf16, tag="e2b")
                if (kind, idx) in SQ_DVE_STEPS:
                    nc.vector.tensor_mul(
                        e2b[:, :, :, 0:L], e_t[:, :, :, 0:L], e_t[:, :, :, 0:L]
                    )
                else:
                    nc.scalar.activation(
                        out=e2b[:, :, :, 0:L], in_=e_t[:, :, :, 0:L], func=AFT.Square
                    )
                for sl in range(2):
                    par, g, rr = rowmap[(step_i, sl)]
                    for c in range(NCH):
                        nc.tensor.matmul(
                            ps[NR * g : NR * (g + 1), 0:L],
                            ohb[:, NR - rr : 2 * NR - rr],
                            e2b[:, sl, c, 0:L],
                            start=False,
                            stop=False,
                            skip_group_check=True,
                        )
            else:
                e2 = e2pool.tile([P, 2, NCH, M], fp8, tag="e2")
                if step_i == 0:
                    for sl in range(2):
                        nc.scalar.activation(
                            out=e2[:, sl, :, 0:L],
                            in_=e_t[:, sl, :, 0:L],
                            func=AFT.Square,
                        )
                else:
                    nc.scalar.activation(
                        out=e2[:, :, :, 0:L], in_=e_t[:, :, :, 0:L], func=AFT.Square
                    )
                for sl in range(2):
                    par, g, rr = rowmap[(step_i, sl)]
                    for c2 in range(NCH // 2):
                        nc.tensor.matmul(
                            ps[NR * g : NR * (g + 1), 0:L],
                            oh8[:, :, NR - rr : 2 * NR - rr],
                            e2[:, sl, 2 * c2 : 2 * c2 + 2, 0:L],
                            start=False,
                            stop=False,
                            skip_group_check=True,
                            perf_mode=mybir.MatmulPerfMode.DoubleRow,
                        )

        for par, (ps, msk) in enumerate(((ps0, mask0), (ps1, mask1))):
            t_m = singles.tile([NPS, M], f32, name=f"t{par}")
            nc.vector.tensor_mul(t_m, ps, msk)
            sq = singles.tile([NPS, M], bf16, name=f"sq{par}")
            res = singles.tile([NPS, 1], f32, name=f"res{par}")
            nc.scalar.activation(out=sq, in_=t_m, func=AFT.Sqrt, accum_out=res)
            nc.sync.dma_start(out=out_d.ap()[par * NPS : (par + 1) * NPS, :], in_=res)

    nc.compile()
    _PROG["nc"] = nc
    return nc


def _slot_infos():
    """Device-order slot list: (parity, psrow, kind, idx, sl)."""
    rowmap = _row_map()
    infos = []
    for step_i, (kind, idx) in enumerate(_emit_order()):
        for sl in range(2):
            par, g, rr = rowmap[(step_i, sl)]
            infos.append((par, NR * g + rr, kind, idx, sl))
    return infos


def _in_maps(repr_np, GT_np):
    import ml_dtypes

    LD_MAX = 320
    r = np.asarray(repr_np, dtype=np.float32)[np.asarray(GT_np).astype(np.int64)]
    rT = np.ascontiguousarray(r.T)  # [N, M] f32
    rT_bf = rT.astype(ml_dtypes.bfloat16)

    rt = np.ascontiguousarray(
        np.transpose(rT_bf.reshape(NCH, P, M), (1, 0, 2))
    ).reshape(P, -1)
    rT_f32_p = np.transpose(rT.reshape(NCH, P, M), (1, 0, 2))  # [P, NCH, M] f32

    infos = _slot_infos()
    maps = []
    for c in range(NCORES):
        js_a = []
        for m in range(NMA):
            js_a += [16 * m + c, 16 * m + 15 - c]
        bias = np.ascontiguousarray(rT_f32_p[:, :, js_a].astype(np.float32)).reshape(
            P, -1
        )

        rtab = np.full((2, N, LD_MAX), HUGE, dtype=np.float32)
        for sl, delta in enumerate((c + 1, 16 - c)):
            shift = TMIN + delta
            take = min(M - shift, LD_MAX)
            rtab[sl, :, :take] = rT[:, shift : shift + take]
        rtab_bf = rtab.astype(ml_dtypes.bfloat16)
        rtab_p = np.transpose(rtab_bf.reshape(2, NCH, P, LD_MAX), (2, 0, 1, 3))
        rtab_p = np.ascontiguousarray(rtab_p).reshape(P, -1)

        mask = np.zeros((2 * NPS, M), dtype=np.float32)
        for par, psrow, kind, idx, sl in infos:
            mrow = par * NPS + psrow
            if kind == "A":
                m = idx
                L = M - 16 * m
                j = 16 * m + c if sl == 0 else 16 * m + 15 - c
                mask[mrow, j - 16 * m + 1 : L] = 1.0
            else:
                k = idx
                L = LD_MAX - 16 * k
                mask[mrow, 0:L] = 1.0

        maps.append({"rt": rt, "rtab": rtab_p, "bias": bias, "mask": mask})
    return maps


def run_device(repr_np, GT_np, trace=False, trace_cores=None):
    """Run the bass kernel on 8 cores; returns (total, BassKernelResults)."""
    from concourse.bass_utils import run_bass_kernel_spmd

    nc = _build_program()
    maps = _in_maps(repr_np, GT_np)
    res = run_bass_kernel_spmd(
        nc,
        maps,
        core_ids=list(range(NCORES)),
        trace=trace,
        trace_cores=trace_cores,
    )
    total = 0.0
    for core_out in res.results:
        total += float(core_out["out"].astype(np.float64).sum())
    return np.float32(total), res


def kernel(repr, GT):
    total, _ = run_device(repr, GT, trace=False)
    return total
